# revision 32
# baseline (speedup 1.0000x reference)
"""ContextualAttentionMask Trainium2 kernel (fp8 DoubleRow version).

Math (per batch sample):
  f: [256, 4096] feature map (channels x pixels), m: [4096] mask
  K[j, :]    = f[:, j] + 1e-7          (per-pixel 1x1 kernel)
  rstd[j]    = 1 / ||K[j, :]||_2
  raw[j, n]  = rstd[j] * sum_c K[c, j] * f[c, n]
  att[j, n]  = softmax_j(raw[j, n])
  fmap[c, n] = sum_j rstd[j] * m[j] * K[j, c] * att[j, n]
  final      = fmap * (1 - m) + f * m  ;  skip branch if mask nearly all-ones

Device computes (per core, unnormalized; host divides, blends, skip-branch):
  E[j, n] = exp(raw[j, n] - 9)   as fp8e5  (bias keeps E in e5m2 range and
                                  ~all of the softmax tail above the e5m2
                                  subnormal floor; cancels in the division)
  o[c, n] = sum_j km8[j, c] * E[j, n]      (km8 = e4m3(rstd * m * K))
  s[n]    = sum_j E[j, n]                  (ones-matmul on PE)

All three matmul families run fp8 with DoubleRow (2 contraction rows per
partition): scores contract ch=256 as 128x2, output/sum contract j in
pairs of 128-blocks. rstd is folded into the scores lhsT on the host so
the exp needs only a constant bias -> ACT instructions can span j-block
pairs ([128, 1024]) without per-row scale vectors.

Sharding: 8 cores = 4 samples x 2 column-halves (2048 columns each).
"""

import sys
from contextlib import ExitStack

import numpy as np

sys.path.insert(0, "/opt/trn_rl_repo")

import ml_dtypes  # noqa: E402

from concourse import bacc, mybir, tile  # noqa: E402
from concourse.bass_utils import run_bass_kernel_spmd  # noqa: E402

FP32 = mybir.dt.float32
FP16 = mybir.dt.float16
I32 = mybir.dt.int32
FP8E4 = mybir.dt.float8e4
FP8E5 = mybir.dt.float8e5
E4 = ml_dtypes.float8_e4m3
E5 = ml_dtypes.float8_e5m2

CH = 256          # channels
J = 4096          # number of per-pixel kernels (= h*w)
NH = 2048         # columns handled per core (half of a sample)
EXP_BIAS = -9.0   # exp(x - 9) keeps E in fp8e5 range; cancels on host
DR = mybir.MatmulPerfMode.DoubleRow

# production configuration (selected by on-hardware A/B benching)
USE_SWI = False
DVE_PAIRS = frozenset()
TIMING_KW = {}                  # extra kwargs for make_in_maps in test.py
TIMING_KW2 = {}                 # extra kwargs for build_program in test.py


def build_program(ch=CH, j_total=J, n_half=NH, loop_reps=1, bufs_e=5,
                  depth=2, hw_reps=1, dve_pairs=(), gps_copy=False,
                  ablate=(), swi=False):
    """ablate: timing-experiment switches ('ones', 'out', 'act', 'scores')
    that drop parts of the pipeline (results become wrong).
    swi: use DoubleRowSwInterleave (host pre-interleaved weights, contiguous
    LDWEIGHTS stream) instead of DoubleRow."""
    return _build(ch, j_total, n_half, loop_reps, bufs_e, depth, hw_reps,
                  frozenset(dve_pairs), gps_copy, frozenset(ablate), swi)


def _build(ch, j_total, n_half, loop_reps, bufs_e, depth, hw_reps,
           dve_pairs, gps_copy, ablate, swi=False):
    """Emit the per-core Bass/Tile program (SPMD across 8 cores).

    loop_reps: python-unrolled repetitions of the main loop (timing only).
    hw_reps: hardware-loop (For_i) repetitions of the main loop (timing
      only; cheap to compile, so thousands of reps are practical).
    dve_pairs: pair indices whose exp runs on the (otherwise idle) DVE via
      the Schraudolph bit trick instead of ACT — offloads the bottleneck
      engine. Safe here: each column's E feeds both o and s from the same
      tile, so the ~3% fast-exp error on the dominant softmax diagonal
      cancels in the o/s division.
    """
    n_pair = j_total // 256   # j-block pairs (DoubleRow granularity)
    qs = 512                  # output column chunk width (one PSUM bank)
    nq = n_half // qs

    nc = bacc.Bacc("TRN2", target_bir_lowering=False, debug=False, num_devices=8)

    n_jb = j_total // 128
    fs_shape = [128, n_jb, 2, 128] if swi else [128, 2, j_total]
    km_shape = ([128, n_pair, 2, 2, 128] if swi
                else [128, n_pair, 2, ch])
    fs_d = nc.dram_tensor("fs8", fs_shape, FP8E4, kind="ExternalInput").ap()
    f_d = nc.dram_tensor("f8", [128, 2, n_half], FP8E4, kind="ExternalInput").ap()
    km_d = nc.dram_tensor("km8", km_shape, FP8E4, kind="ExternalInput").ap()
    o_d = nc.dram_tensor("o", [ch, n_half], FP16, kind="ExternalOutput").ap()
    s_d = nc.dram_tensor("s", [1, n_half], FP32, kind="ExternalOutput").ap()

    with tile.TileContext(nc) as tc, ExitStack() as ctx:
        const_p = ctx.enter_context(tc.tile_pool(name="const", bufs=1))
        fs_p = ctx.enter_context(tc.tile_pool(name="fs", bufs=1))
        f8_p = ctx.enter_context(tc.tile_pool(name="f8", bufs=1))
        km_p = ctx.enter_context(tc.tile_pool(name="km", bufs=1))
        e_p = ctx.enter_context(tc.tile_pool(name="e", bufs=bufs_e))
        osb_p = ctx.enter_context(tc.tile_pool(name="osb", bufs=3))
        ssb_p = ctx.enter_context(tc.tile_pool(name="ssb", bufs=2))
        ps_sc = ctx.enter_context(
            tc.tile_pool(name="ps_sc", bufs=2, space="PSUM"))
        ps_out = ctx.enter_context(
            tc.tile_pool(name="ps_out", bufs=4, space="PSUM"))
        ei_p = (ctx.enter_context(tc.tile_pool(name="ei", bufs=2))
                if dve_pairs else None)

        # Schraudolph fast-exp constants: bits(2^t) ~= (t + 127 - c) * 2^23
        FEXP_MUL = 1.4426950408889634 * 8388608.0
        FEXP_ADD = (EXP_BIAS * 1.4426950408889634 + 127.0 - 0.04303) * 8388608.0

        # DoubleRow ldweights needs pair-dim byte-stride % 16 == 0
        ones8 = const_p.tile([128, 2, 16], FP8E4, tag="ones")
        nc.vector.memset(ones8[:], 1.0)
        bias_e = const_p.tile([128, 1], FP32, tag="bias_e")
        nc.vector.memset(bias_e[:], EXP_BIAS)
        # dummy exp: pulls the ACT exp-table load into the DMA window
        warm = const_p.tile([128, 1], FP32, tag="warm")
        nc.scalar.activation(warm[:], bias_e[:],
                             mybir.ActivationFunctionType.Exp)

        fs = fs_p.tile(fs_shape, FP8E4, tag="fs")
        f8 = f8_p.tile([128, 2, n_half], FP8E4, tag="f8")
        km = km_p.tile(km_shape, FP8E4, tag="km")
        PM = mybir.MatmulPerfMode.DoubleRowSwInterleave if swi else DR

        def fs_w(jb):
            return fs[:, jb] if swi else fs[:, :, jb * 128:(jb + 1) * 128]

        def km_w(p0, cb):
            return (km[:, p0, cb] if swi
                    else km[:, p0, :, cb * 128:(cb + 1) * 128])
        e_static = None
        if "act" in ablate:
            e_static = const_p.tile([128, 2, qs], FP8E5, tag="e_static")
            nc.vector.memset(e_static[:], 0.001)

        # input DMA ordered by first-use time; the first critical transfers
        # go out on separate engine queues so their DGE setups overlap
        def fs_dma(eng, jb0, jb1):
            if swi:
                eng.dma_start(out=fs[:, jb0:jb1], in_=fs_d[:, jb0:jb1])
            else:
                eng.dma_start(out=fs[:, :, jb0 * 128:jb1 * 128],
                              in_=fs_d[:, :, jb0 * 128:jb1 * 128])

        nc.sync.dma_start(out=f8[:, :, 0:qs], in_=f_d[:, :, 0:qs])
        fs_dma(nc.gpsimd, 0, 4)
        nc.scalar.dma_start(out=km[:, 0:2], in_=km_d[:, 0:2])
        fs_dma(nc.sync, 4, 8)
        nc.gpsimd.dma_start(out=km[:, 2:6], in_=km_d[:, 2:6])
        fs_dma(nc.sync, 8, 16)
        fs_dma(nc.gpsimd, 16, n_jb)
        nc.gpsimd.dma_start(out=km[:, 6:n_pair], in_=km_d[:, 6:n_pair])
        for q in range(1, nq):
            nc.sync.dma_start(out=f8[:, :, q * qs:(q + 1) * qs],
                              in_=f_d[:, :, q * qs:(q + 1) * qs])

        # fused main loop: scores -> exp -> {Km^T E, 1^T E} accumulation.
        # Software pipeline: exp-dependent matmuls trail the score matmuls
        # by `depth` pairs so the in-order PE queue never waits on ACT.
        import contextlib
        loop_ctx = (tc.For_i(0, hw_reps) if hw_reps > 1
                    else contextlib.nullcontext())
        with loop_ctx:
         for q in [qq for _ in range(loop_reps) for qq in range(nq)]:
            nsl = slice(q * qs, (q + 1) * qs)
            out_ps = [
                ps_out.tile([128, qs], FP32, tag="out", name=f"out_ps{cb}")
                for cb in range(2)
            ]
            sum_ps = ps_out.tile([1, qs], FP32, tag="out", name="sum_ps")
            etiles = {}
            for pp in range(n_pair + depth):
                if pp < n_pair:
                    ps = ps_sc.tile([128, 2, qs], FP32, tag="sc", name="ps")
                    if "scores" not in ablate:
                        for i in range(2):
                            jb = pp * 2 + i
                            nc.tensor.matmul(
                                ps[:, i, :], fs_w(jb), f8[:, :, nsl],
                                start=True, stop=True, perf_mode=PM,
                            )
                    if "act" in ablate:
                        etiles[pp] = e_static
                    elif pp in dve_pairs:
                        e = e_p.tile([128, 2, qs], FP8E5, tag="e", name="e")
                        ei = ei_p.tile([128, 2, qs], I32, tag="ei", name="ei")
                        nc.vector.tensor_scalar(
                            ei[:], ps[:], FEXP_MUL, FEXP_ADD,
                            mybir.AluOpType.mult, mybir.AluOpType.add,
                        )
                        cpeng = nc.gpsimd if gps_copy else nc.vector
                        cpeng.tensor_copy(e[:], ei[:].bitcast(FP32))
                        etiles[pp] = e
                    else:
                        e = e_p.tile([128, 2, qs], FP8E5, tag="e", name="e")
                        nc.scalar.activation(
                            e[:], ps[:], mybir.ActivationFunctionType.Exp,
                            bias=bias_e[:],
                        )
                        etiles[pp] = e
                if pp >= depth:
                    p0 = pp - depth
                    e = etiles.pop(p0)
                    if "out" not in ablate:
                        for cb in range(2):
                            nc.tensor.matmul(
                                out_ps[cb][:], km_w(p0, cb), e[:],
                                start=(p0 == 0), stop=(p0 == n_pair - 1),
                                perf_mode=PM,
                            )
                    if "ones" not in ablate:
                        # ones stays plain DoubleRow: DRS rejects 1-col
                        # weights (s3_lw_valid_num_active_cols); the moving
                        # operand layout is identical in both modes.
                        nc.tensor.matmul(
                            sum_ps[:], ones8[:, :, 0:1], e[:],
                            start=(p0 == 0), stop=(p0 == n_pair - 1),
                            perf_mode=DR,
                        )
            last = q == nq - 1
            if "ones" not in ablate:
                srow = ssb_p.tile([1, qs], FP32, tag="srow", name="srow")
                nc.vector.tensor_copy(srow[:], sum_ps[:])
                (nc.gpsimd if last else nc.sync).dma_start(
                    out=s_d[0:1, nsl], in_=srow[:])
            for cb in range(2 if "out" not in ablate else 0):
                osb = osb_p.tile([128, qs], FP16, tag="osb", name="osb")
                if last and cb == 0:
                    # ACT is idle after its final exp; splitting the drain
                    # copies between ACT and DVE shortens the program tail
                    nc.scalar.copy(osb[:], out_ps[cb][:])
                else:
                    nc.vector.tensor_copy(osb[:], out_ps[cb][:])
                eng = (nc.scalar if cb == 0 else nc.sync) if last else nc.sync
                eng.dma_start(out=o_d[cb * 128:(cb + 1) * 128, nsl], in_=osb[:])

    nc.compile()
    return nc


_CACHE = {}


def _get_program():
    if "nc" not in _CACHE:
        _CACHE["nc"] = build_program(swi=USE_SWI, dve_pairs=DVE_PAIRS)
    return _CACHE["nc"]


def _get_runner():
    """Cached sharded executable over 8 cores (same program/plugin as
    run_bass_kernel_spmd's axon path, but without per-call retracing)."""
    if "runner" in _CACHE:
        return _CACHE["runner"]
    import jax
    from jax.sharding import Mesh, NamedSharding, PartitionSpec
    from jax.experimental.shard_map import shard_map
    from concourse import bass2jax, mybir
    from concourse.bass2jax import _bass_exec_p, partition_id_tensor

    nc = _get_program()
    bass2jax.install_neuronx_cc_hook()
    pname = nc.partition_id_tensor.name if nc.partition_id_tensor else None

    in_names, out_names, out_avals = [], [], []
    for alloc in nc.m.functions[0].allocations:
        if not isinstance(alloc, mybir.MemoryLocationSet):
            continue
        name = alloc.memorylocations[0].name
        if alloc.kind == "ExternalInput":
            if name != pname:
                in_names.append(name)
        elif alloc.kind == "ExternalOutput":
            out_names.append(name)
            out_avals.append(
                jax.core.ShapedArray(
                    tuple(alloc.tensor_shape), mybir.dt.np(alloc.dtype)
                )
            )
    n_params, n_outs = len(in_names), len(out_names)
    all_in = in_names + out_names + ([pname] if pname else [])

    def _body(*args):
        operands = list(args)
        if pname is not None:
            operands.append(partition_id_tensor())
        return tuple(_bass_exec_p.bind(
            *operands, out_avals=tuple(out_avals), in_names=tuple(all_in),
            out_names=tuple(out_names), lowering_input_output_aliases=(),
            sim_require_finite=True, sim_require_nnan=True, nc=nc,
        ))

    devices = jax.devices()[:8]
    mesh = Mesh(np.asarray(devices), ("core",))
    spec = NamedSharding(mesh, PartitionSpec("core"))
    fn = jax.jit(
        shard_map(
            _body, mesh=mesh,
            in_specs=(PartitionSpec("core"),) * (n_params + n_outs),
            out_specs=(PartitionSpec("core"),) * n_outs,
            check_rep=False,
        ),
        donate_argnums=tuple(range(n_params, n_params + n_outs)),
        keep_unused=True,
    )
    zero_host = [
        np.zeros((8 * a.shape[0], *a.shape[1:]), a.dtype) for a in out_avals
    ]

    def run(in_maps):
        concat_in = [
            np.concatenate([np.asarray(m[name]) for m in in_maps], axis=0)
            for name in in_names
        ]
        zeros = [jax.device_put(z, spec) for z in zero_host]
        out = fn(*concat_in, *zeros)
        return [
            {
                name: np.asarray(out[i]).reshape(8, *out_avals[i].shape)[c]
                for i, name in enumerate(out_names)
            }
            for c in range(8)
        ]

    _CACHE["runner"] = run
    return run


def _drs_pack(W):
    """SwInterleave weight packing: logical [..., 2, F] -> flat layout with
    A/B pairs interleaved per column and columns reversed (HW-verified)."""
    return np.ascontiguousarray(
        np.flip(W, axis=-1).swapaxes(-1, -2)).reshape(W.shape)


def make_in_maps(foreground, mask, swi=False):
    """Per-core host-side input prep: fp8 casts + DoubleRow layouts."""
    bs, ch, h, w = foreground.shape
    hw = h * w
    half = hw // 2
    f = np.ascontiguousarray(foreground.reshape(bs, ch, hw), dtype=np.float32)
    m = np.ascontiguousarray(mask.reshape(bs, hw), dtype=np.float32)
    in_maps = []
    for b in range(bs):
        k = f[b] + np.float32(1e-7)                 # [ch, hw], reference's +1e-7
        rstd = 1.0 / np.sqrt((k * k).sum(axis=0, dtype=np.float64))  # [hw]
        rstd = rstd.astype(np.float32)
        # scores lhsT: rstd folded in; [128, 2, hw] ch-pair layout
        fs8 = (rstd[None, :] * k).reshape(2, 128, hw).transpose(1, 0, 2)
        fs8 = np.ascontiguousarray(fs8).astype(E4)
        # out lhsT: km[j, c] = rstd_j m_j K[j, c]; [128, pairs, 2, ch]
        km_full = ((rstd * m[b])[:, None] * k.T)     # [hw, ch]
        km8 = km_full.reshape(hw // 256, 2, 128, ch).transpose(2, 0, 1, 3)
        km8 = np.ascontiguousarray(km8).astype(E4)
        if swi:
            # fs: [128, 2, hw] -> per-jb packed [128, n_jb, 2, 128]
            fs8 = _drs_pack(
                fs8.reshape(128, 2, hw // 128, 128).transpose(0, 2, 1, 3)
            )
            # km: [128, pairs, 2, ch] -> per-(pair, cb) packed
            # [128, pairs, cb, 2, 128]
            km8 = _drs_pack(
                km8.reshape(128, hw // 256, 2, 2, 128).transpose(0, 1, 3, 2, 4)
            )
        for hh in range(2):
            fh = f[b][:, hh * half:(hh + 1) * half]  # [ch, half]
            f8 = fh.reshape(2, 128, half).transpose(1, 0, 2)
            in_maps.append({
                "fs8": fs8,
                "f8": np.ascontiguousarray(f8).astype(E4),
                "km8": km8,
            })
    return in_maps


def kernel(foreground, mask):
    foreground = np.asarray(foreground, dtype=np.float32)
    mask = np.asarray(mask, dtype=np.float32)
    bs, ch, h, w = foreground.shape
    hw = h * w

    in_maps = make_in_maps(foreground, mask, swi=USE_SWI)
    try:
        results = _get_runner()(in_maps)
    except Exception:
        # robust fallback: the generic SPMD entry point
        res = run_bass_kernel_spmd(_get_program(), in_maps, list(range(8)))
        results = res.results

    fmap = np.empty((bs, ch, h, w), dtype=np.float32)
    rows = h // 2
    for core in range(8):
        b, hh = core // 2, core % 2
        o = results[core]["o"]       # [ch, hw/2] unnormalized
        s = results[core]["s"]       # [1, hw/2] softmax denominator
        fmap[b, :, hh * rows:(hh + 1) * rows, :] = (o / s).reshape(ch, rows, w)

    mm = mask[:, 0:1]                    # [bs, 1, h, w]
    final = fmap * (1.0 - mm) + foreground * mm
    skip = mask.sum(axis=(1, 2, 3)) > (hw - 10)
    final[skip] = foreground[skip]
    return final.astype(np.float32)


# revision 36
# speedup vs baseline: 1.7344x; 1.7344x over previous
"""ContextualAttentionMask Trainium2 kernel (fp8 DoubleRow version).

Math (per batch sample):
  f: [256, 4096] feature map (channels x pixels), m: [4096] mask
  K[j, :]    = f[:, j] + 1e-7          (per-pixel 1x1 kernel)
  rstd[j]    = 1 / ||K[j, :]||_2
  raw[j, n]  = rstd[j] * sum_c K[c, j] * f[c, n]
  att[j, n]  = softmax_j(raw[j, n])
  fmap[c, n] = sum_j rstd[j] * m[j] * K[j, c] * att[j, n]
  final      = fmap * (1 - m) + f * m  ;  skip branch if mask nearly all-ones

Device computes (per core, unnormalized; host divides, blends, skip-branch):
  E[j, n] = exp(raw[j, n] - 9)   as fp8e5  (bias keeps E in e5m2 range and
                                  ~all of the softmax tail above the e5m2
                                  subnormal floor; cancels in the division)
  o[c, n] = sum_j km8[j, c] * E[j, n]      (km8 = e4m3(rstd * m * K))
  s[n]    = sum_j E[j, n]                  (ones-matmul on PE)

All three matmul families run fp8 with DoubleRow (2 contraction rows per
partition): scores contract ch=256 as 128x2, output/sum contract j in
pairs of 128-blocks. rstd is folded into the scores lhsT on the host so
the exp needs only a constant bias -> ACT instructions can span j-block
pairs ([128, 1024]) without per-row scale vectors.

Sharding: 8 cores = 4 samples x 2 column-halves (2048 columns each).
"""

import sys
from contextlib import ExitStack

import numpy as np

sys.path.insert(0, "/opt/trn_rl_repo")

import ml_dtypes  # noqa: E402

from concourse import bacc, mybir, tile  # noqa: E402
from concourse.bass_utils import run_bass_kernel_spmd  # noqa: E402

FP32 = mybir.dt.float32
FP16 = mybir.dt.float16
I32 = mybir.dt.int32
FP8E4 = mybir.dt.float8e4
FP8E5 = mybir.dt.float8e5
E4 = ml_dtypes.float8_e4m3
E5 = ml_dtypes.float8_e5m2

CH = 256          # channels
J = 4096          # number of per-pixel kernels (= h*w)
NH = 2048         # columns handled per core (half of a sample)
EXP_BIAS = -9.0   # exp(x - 9) keeps E in fp8e5 range; cancels on host
DR = mybir.MatmulPerfMode.DoubleRow

# production configuration (selected by on-hardware A/B benching)
USE_SWI = False
DVE_PAIRS = frozenset()
DEPTH = 4                       # software-pipeline lead of scores over out
BUFS_E = 6
TIMING_KW = {}                  # extra kwargs for make_in_maps in test.py
TIMING_KW2 = {}                 # extra kwargs for build_program in test.py


def build_program(ch=CH, j_total=J, n_half=NH, loop_reps=1, bufs_e=None,
                  depth=None, hw_reps=1, dve_pairs=(), gps_copy=False,
                  ablate=(), swi=False):
    """ablate: timing-experiment switches ('ones', 'out', 'act', 'scores')
    that drop parts of the pipeline (results become wrong).
    swi: use DoubleRowSwInterleave (host pre-interleaved weights, contiguous
    LDWEIGHTS stream) instead of DoubleRow."""
    if depth is None:
        depth = DEPTH
    if bufs_e is None:
        bufs_e = max(BUFS_E, depth + 2)
    return _build(ch, j_total, n_half, loop_reps, bufs_e, depth, hw_reps,
                  frozenset(dve_pairs), gps_copy, frozenset(ablate), swi)


def _build(ch, j_total, n_half, loop_reps, bufs_e, depth, hw_reps,
           dve_pairs, gps_copy, ablate, swi=False):
    """Emit the per-core Bass/Tile program (SPMD across 8 cores).

    loop_reps: python-unrolled repetitions of the main loop (timing only).
    hw_reps: hardware-loop (For_i) repetitions of the main loop (timing
      only; cheap to compile, so thousands of reps are practical).
    dve_pairs: pair indices whose exp runs on the (otherwise idle) DVE via
      the Schraudolph bit trick instead of ACT — offloads the bottleneck
      engine. Safe here: each column's E feeds both o and s from the same
      tile, so the ~3% fast-exp error on the dominant softmax diagonal
      cancels in the o/s division.
    """
    n_pair = j_total // 256   # j-block pairs (DoubleRow granularity)
    qs = 512                  # output column chunk width (one PSUM bank)
    nq = n_half // qs

    nc = bacc.Bacc("TRN2", target_bir_lowering=False, debug=False, num_devices=8)

    n_jb = j_total // 128
    fs_shape = [128, n_jb, 2, 128] if swi else [128, 2, j_total]
    km_shape = ([128, n_pair, 2, 2, 128] if swi
                else [128, n_pair, 2, ch])
    fs_d = nc.dram_tensor("fs8", fs_shape, FP8E4, kind="ExternalInput").ap()
    f_d = nc.dram_tensor("f8", [128, 2, n_half], FP8E4, kind="ExternalInput").ap()
    km_d = nc.dram_tensor("km8", km_shape, FP8E4, kind="ExternalInput").ap()
    o_d = nc.dram_tensor("o", [ch, n_half], FP16, kind="ExternalOutput").ap()
    s_d = nc.dram_tensor("s", [1, n_half], FP32, kind="ExternalOutput").ap()

    with tile.TileContext(nc) as tc, ExitStack() as ctx:
        const_p = ctx.enter_context(tc.tile_pool(name="const", bufs=1))
        fs_p = ctx.enter_context(tc.tile_pool(name="fs", bufs=1))
        f8_p = ctx.enter_context(tc.tile_pool(name="f8", bufs=1))
        km_p = ctx.enter_context(tc.tile_pool(name="km", bufs=1))
        e_p = ctx.enter_context(tc.tile_pool(name="e", bufs=bufs_e))
        osb_p = ctx.enter_context(tc.tile_pool(name="osb", bufs=3))
        ssb_p = ctx.enter_context(tc.tile_pool(name="ssb", bufs=2))
        ps_sc = ctx.enter_context(
            tc.tile_pool(name="ps_sc", bufs=2, space="PSUM"))
        ps_out = ctx.enter_context(
            tc.tile_pool(name="ps_out", bufs=4, space="PSUM"))
        ei_p = (ctx.enter_context(tc.tile_pool(name="ei", bufs=2))
                if dve_pairs else None)

        # Schraudolph fast-exp constants: bits(2^t) ~= (t + 127 - c) * 2^23
        FEXP_MUL = 1.4426950408889634 * 8388608.0
        FEXP_ADD = (EXP_BIAS * 1.4426950408889634 + 127.0 - 0.04303) * 8388608.0

        # DoubleRow ldweights needs pair-dim byte-stride % 16 == 0
        ones8 = const_p.tile([128, 2, 16], FP8E4, tag="ones")
        nc.vector.memset(ones8[:], 1.0)
        bias_e = const_p.tile([128, 1], FP32, tag="bias_e")
        nc.vector.memset(bias_e[:], EXP_BIAS)
        # dummy exp: pulls the ACT exp-table load into the DMA window
        warm = const_p.tile([128, 1], FP32, tag="warm")
        nc.scalar.activation(warm[:], bias_e[:],
                             mybir.ActivationFunctionType.Exp)

        fs = fs_p.tile(fs_shape, FP8E4, tag="fs")
        f8 = f8_p.tile([128, 2, n_half], FP8E4, tag="f8")
        km = km_p.tile(km_shape, FP8E4, tag="km")
        PM = mybir.MatmulPerfMode.DoubleRowSwInterleave if swi else DR

        def fs_w(jb):
            return fs[:, jb] if swi else fs[:, :, jb * 128:(jb + 1) * 128]

        def km_w(p0, cb):
            return (km[:, p0, cb] if swi
                    else km[:, p0, :, cb * 128:(cb + 1) * 128])
        e_static = None
        if "act" in ablate:
            e_static = const_p.tile([128, 2, qs], FP8E5, tag="e_static")
            nc.vector.memset(e_static[:], 0.001)

        # input DMA ordered by first-use time; the first critical transfers
        # go out on separate engine queues so their DGE setups overlap
        def fs_dma(eng, jb0, jb1):
            if swi:
                eng.dma_start(out=fs[:, jb0:jb1], in_=fs_d[:, jb0:jb1])
            else:
                eng.dma_start(out=fs[:, :, jb0 * 128:jb1 * 128],
                              in_=fs_d[:, :, jb0 * 128:jb1 * 128])

        nc.sync.dma_start(out=f8[:, :, 0:qs], in_=f_d[:, :, 0:qs])
        fs_dma(nc.gpsimd, 0, 4)
        nc.scalar.dma_start(out=km[:, 0:2], in_=km_d[:, 0:2])
        fs_dma(nc.sync, 4, 8)
        nc.gpsimd.dma_start(out=km[:, 2:6], in_=km_d[:, 2:6])
        fs_dma(nc.sync, 8, 16)
        fs_dma(nc.gpsimd, 16, n_jb)
        nc.gpsimd.dma_start(out=km[:, 6:n_pair], in_=km_d[:, 6:n_pair])
        for q in range(1, nq):
            nc.sync.dma_start(out=f8[:, :, q * qs:(q + 1) * qs],
                              in_=f_d[:, :, q * qs:(q + 1) * qs])

        # fused main loop, globally software-pipelined: score matmuls and
        # exp lead the e-dependent (out/ones) matmuls by `depth` pairs
        # ACROSS chunk boundaries, so the in-order PE queue never drains
        # at a chunk flush and ACT stays fed continuously.
        import contextlib
        loop_ctx = (tc.For_i(0, hw_reps) if hw_reps > 1
                    else contextlib.nullcontext())
        with loop_ctx:
            gps = [(qq, pp) for _ in range(loop_reps)
                   for qq in range(nq) for pp in range(n_pair)]
            N = len(gps)
            etiles = {}
            cur_out = {}
            for idx in range(N + depth):
                if idx < N:
                    q, pp = gps[idx]
                    nsl = slice(q * qs, (q + 1) * qs)
                    ps = ps_sc.tile([128, 2, qs], FP32, tag="sc", name="ps")
                    if "scores" not in ablate:
                        for i in range(2):
                            jb = pp * 2 + i
                            nc.tensor.matmul(
                                ps[:, i, :], fs_w(jb), f8[:, :, nsl],
                                start=True, stop=True, perf_mode=PM,
                            )
                    if "act" in ablate:
                        etiles[idx] = e_static
                    elif pp in dve_pairs:
                        e = e_p.tile([128, 2, qs], FP8E5, tag="e", name="e")
                        ei = ei_p.tile([128, 2, qs], I32, tag="ei", name="ei")
                        nc.vector.tensor_scalar(
                            ei[:], ps[:], FEXP_MUL, FEXP_ADD,
                            mybir.AluOpType.mult, mybir.AluOpType.add,
                        )
                        cpeng = nc.gpsimd if gps_copy else nc.vector
                        cpeng.tensor_copy(e[:], ei[:].bitcast(FP32))
                        etiles[idx] = e
                    else:
                        e = e_p.tile([128, 2, qs], FP8E5, tag="e", name="e")
                        nc.scalar.activation(
                            e[:], ps[:], mybir.ActivationFunctionType.Exp,
                            bias=bias_e[:],
                        )
                        etiles[idx] = e
                if idx >= depth:
                    jdx = idx - depth
                    q2, p2 = gps[jdx]
                    nsl2 = slice(q2 * qs, (q2 + 1) * qs)
                    if p2 == 0:
                        cur_out = {
                            "out": [
                                ps_out.tile([128, qs], FP32, tag="out",
                                            name=f"out_ps{cb}")
                                for cb in range(2)
                            ],
                            "sum": ps_out.tile([1, qs], FP32, tag="out",
                                               name="sum_ps"),
                        }
                    e = etiles.pop(jdx)
                    if "out" not in ablate:
                        for cb in range(2):
                            nc.tensor.matmul(
                                cur_out["out"][cb][:], km_w(p2, cb), e[:],
                                start=(p2 == 0), stop=(p2 == n_pair - 1),
                                perf_mode=PM,
                            )
                    if "ones" not in ablate:
                        # ones stays plain DoubleRow: DRS rejects 1-col
                        # weights (s3_lw_valid_num_active_cols); the moving
                        # operand layout is identical in both modes.
                        nc.tensor.matmul(
                            cur_out["sum"][:], ones8[:, :, 0:1], e[:],
                            start=(p2 == 0), stop=(p2 == n_pair - 1),
                            perf_mode=DR,
                        )
                    if p2 == n_pair - 1:
                        last = jdx == N - 1
                        if "ones" not in ablate:
                            srow = ssb_p.tile([1, qs], FP32, tag="srow",
                                              name="srow")
                            nc.vector.tensor_copy(srow[:], cur_out["sum"][:])
                            (nc.gpsimd if last else nc.sync).dma_start(
                                out=s_d[0:1, nsl2], in_=srow[:])
                        for cb in range(2 if "out" not in ablate else 0):
                            osb = osb_p.tile([128, qs], FP16, tag="osb",
                                             name="osb")
                            if last and cb == 0:
                                # ACT is idle after its final exp; splitting
                                # the drain copies between ACT and DVE
                                # shortens the program tail
                                nc.scalar.copy(osb[:], cur_out["out"][cb][:])
                            else:
                                nc.vector.tensor_copy(
                                    osb[:], cur_out["out"][cb][:])
                            eng = ((nc.scalar if cb == 0 else nc.sync)
                                   if last else nc.sync)
                            eng.dma_start(
                                out=o_d[cb * 128:(cb + 1) * 128, nsl2],
                                in_=osb[:])

    nc.compile()
    return nc


_CACHE = {}


def _get_program():
    if "nc" not in _CACHE:
        _CACHE["nc"] = build_program(swi=USE_SWI, dve_pairs=DVE_PAIRS)
    return _CACHE["nc"]


def _get_runner():
    """Cached sharded executable over 8 cores (same program/plugin as
    run_bass_kernel_spmd's axon path, but without per-call retracing)."""
    if "runner" in _CACHE:
        return _CACHE["runner"]
    import jax
    from jax.sharding import Mesh, NamedSharding, PartitionSpec
    from jax.experimental.shard_map import shard_map
    from concourse import bass2jax, mybir
    from concourse.bass2jax import _bass_exec_p, partition_id_tensor

    nc = _get_program()
    bass2jax.install_neuronx_cc_hook()
    pname = nc.partition_id_tensor.name if nc.partition_id_tensor else None

    in_names, out_names, out_avals = [], [], []
    for alloc in nc.m.functions[0].allocations:
        if not isinstance(alloc, mybir.MemoryLocationSet):
            continue
        name = alloc.memorylocations[0].name
        if alloc.kind == "ExternalInput":
            if name != pname:
                in_names.append(name)
        elif alloc.kind == "ExternalOutput":
            out_names.append(name)
            out_avals.append(
                jax.core.ShapedArray(
                    tuple(alloc.tensor_shape), mybir.dt.np(alloc.dtype)
                )
            )
    n_params, n_outs = len(in_names), len(out_names)
    all_in = in_names + out_names + ([pname] if pname else [])

    def _body(*args):
        operands = list(args)
        if pname is not None:
            operands.append(partition_id_tensor())
        return tuple(_bass_exec_p.bind(
            *operands, out_avals=tuple(out_avals), in_names=tuple(all_in),
            out_names=tuple(out_names), lowering_input_output_aliases=(),
            sim_require_finite=True, sim_require_nnan=True, nc=nc,
        ))

    devices = jax.devices()[:8]
    mesh = Mesh(np.asarray(devices), ("core",))
    spec = NamedSharding(mesh, PartitionSpec("core"))
    fn = jax.jit(
        shard_map(
            _body, mesh=mesh,
            in_specs=(PartitionSpec("core"),) * (n_params + n_outs),
            out_specs=(PartitionSpec("core"),) * n_outs,
            check_rep=False,
        ),
        donate_argnums=tuple(range(n_params, n_params + n_outs)),
        keep_unused=True,
    )
    zero_host = [
        np.zeros((8 * a.shape[0], *a.shape[1:]), a.dtype) for a in out_avals
    ]

    def run(in_maps):
        concat_in = [
            np.concatenate([np.asarray(m[name]) for m in in_maps], axis=0)
            for name in in_names
        ]
        zeros = [jax.device_put(z, spec) for z in zero_host]
        out = fn(*concat_in, *zeros)
        return [
            {
                name: np.asarray(out[i]).reshape(8, *out_avals[i].shape)[c]
                for i, name in enumerate(out_names)
            }
            for c in range(8)
        ]

    _CACHE["runner"] = run
    return run


def _drs_pack(W):
    """SwInterleave weight packing: logical [..., 2, F] -> flat layout with
    A/B pairs interleaved per column and columns reversed (HW-verified)."""
    return np.ascontiguousarray(
        np.flip(W, axis=-1).swapaxes(-1, -2)).reshape(W.shape)


def make_in_maps(foreground, mask, swi=False):
    """Per-core host-side input prep: fp8 casts + DoubleRow layouts."""
    bs, ch, h, w = foreground.shape
    hw = h * w
    half = hw // 2
    f = np.ascontiguousarray(foreground.reshape(bs, ch, hw), dtype=np.float32)
    m = np.ascontiguousarray(mask.reshape(bs, hw), dtype=np.float32)
    in_maps = []
    for b in range(bs):
        k = f[b] + np.float32(1e-7)                 # [ch, hw], reference's +1e-7
        rstd = 1.0 / np.sqrt((k * k).sum(axis=0, dtype=np.float64))  # [hw]
        rstd = rstd.astype(np.float32)
        # scores lhsT: rstd folded in; [128, 2, hw] ch-pair layout
        fs8 = (rstd[None, :] * k).reshape(2, 128, hw).transpose(1, 0, 2)
        fs8 = np.ascontiguousarray(fs8).astype(E4)
        # out lhsT: km[j, c] = rstd_j m_j K[j, c]; [128, pairs, 2, ch]
        km_full = ((rstd * m[b])[:, None] * k.T)     # [hw, ch]
        km8 = km_full.reshape(hw // 256, 2, 128, ch).transpose(2, 0, 1, 3)
        km8 = np.ascontiguousarray(km8).astype(E4)
        if swi:
            # fs: [128, 2, hw] -> per-jb packed [128, n_jb, 2, 128]
            fs8 = _drs_pack(
                fs8.reshape(128, 2, hw // 128, 128).transpose(0, 2, 1, 3)
            )
            # km: [128, pairs, 2, ch] -> per-(pair, cb) packed
            # [128, pairs, cb, 2, 128]
            km8 = _drs_pack(
                km8.reshape(128, hw // 256, 2, 2, 128).transpose(0, 1, 3, 2, 4)
            )
        for hh in range(2):
            fh = f[b][:, hh * half:(hh + 1) * half]  # [ch, half]
            f8 = fh.reshape(2, 128, half).transpose(1, 0, 2)
            in_maps.append({
                "fs8": fs8,
                "f8": np.ascontiguousarray(f8).astype(E4),
                "km8": km8,
            })
    return in_maps


def kernel(foreground, mask):
    foreground = np.asarray(foreground, dtype=np.float32)
    mask = np.asarray(mask, dtype=np.float32)
    bs, ch, h, w = foreground.shape
    hw = h * w

    in_maps = make_in_maps(foreground, mask, swi=USE_SWI)
    try:
        results = _get_runner()(in_maps)
    except Exception:
        # robust fallback: the generic SPMD entry point
        res = run_bass_kernel_spmd(_get_program(), in_maps, list(range(8)))
        results = res.results

    fmap = np.empty((bs, ch, h, w), dtype=np.float32)
    rows = h // 2
    for core in range(8):
        b, hh = core // 2, core % 2
        o = results[core]["o"]       # [ch, hw/2] unnormalized
        s = results[core]["s"]       # [1, hw/2] softmax denominator
        fmap[b, :, hh * rows:(hh + 1) * rows, :] = (o / s).reshape(ch, rows, w)

    mm = mask[:, 0:1]                    # [bs, 1, h, w]
    final = fmap * (1.0 - mm) + foreground * mm
    skip = mask.sum(axis=(1, 2, 3)) > (hw - 10)
    final[skip] = foreground[skip]
    return final.astype(np.float32)


# revision 38
# speedup vs baseline: 2.0080x; 1.1577x over previous
"""ContextualAttentionMask Trainium2 kernel (fp8 DoubleRow version).

Math (per batch sample):
  f: [256, 4096] feature map (channels x pixels), m: [4096] mask
  K[j, :]    = f[:, j] + 1e-7          (per-pixel 1x1 kernel)
  rstd[j]    = 1 / ||K[j, :]||_2
  raw[j, n]  = rstd[j] * sum_c K[c, j] * f[c, n]
  att[j, n]  = softmax_j(raw[j, n])
  fmap[c, n] = sum_j rstd[j] * m[j] * K[j, c] * att[j, n]
  final      = fmap * (1 - m) + f * m  ;  skip branch if mask nearly all-ones

Device computes (per core, unnormalized; host divides, blends, skip-branch):
  E[j, n] = exp(raw[j, n] - 9)   as fp8e5  (bias keeps E in e5m2 range and
                                  ~all of the softmax tail above the e5m2
                                  subnormal floor; cancels in the division)
  o[c, n] = sum_j km8[j, c] * E[j, n]      (km8 = e4m3(rstd * m * K))
  s[n]    = sum_j E[j, n]                  (ones-matmul on PE)

All three matmul families run fp8 with DoubleRow (2 contraction rows per
partition): scores contract ch=256 as 128x2, output/sum contract j in
pairs of 128-blocks. rstd is folded into the scores lhsT on the host so
the exp needs only a constant bias -> ACT instructions can span j-block
pairs ([128, 1024]) without per-row scale vectors.

Sharding: 8 cores = 4 samples x 2 column-halves (2048 columns each).
"""

import sys
from contextlib import ExitStack

import numpy as np

sys.path.insert(0, "/opt/trn_rl_repo")

import ml_dtypes  # noqa: E402

from concourse import bacc, mybir, tile  # noqa: E402
from concourse.bass_utils import run_bass_kernel_spmd  # noqa: E402

FP32 = mybir.dt.float32
FP16 = mybir.dt.float16
I32 = mybir.dt.int32
FP8E4 = mybir.dt.float8e4
FP8E5 = mybir.dt.float8e5
E4 = ml_dtypes.float8_e4m3
E5 = ml_dtypes.float8_e5m2

CH = 256          # channels
J = 4096          # number of per-pixel kernels (= h*w)
NH = 2048         # columns handled per core (half of a sample)
EXP_BIAS = -9.0   # exp(x - 9) keeps E in fp8e5 range; cancels on host
DR = mybir.MatmulPerfMode.DoubleRow

# production configuration (selected by on-hardware A/B benching)
USE_SWI = False
DVE_PAIRS = frozenset()
DEPTH = 4                       # software-pipeline lead of scores over out
BUFS_E = 6
S_SPLIT = True                  # odd pairs' s-accumulation on DVE (PE relief)
TIMING_KW = {}                  # extra kwargs for make_in_maps in test.py
TIMING_KW2 = {}                 # extra kwargs for build_program in test.py


def build_program(ch=CH, j_total=J, n_half=NH, loop_reps=1, bufs_e=None,
                  depth=None, hw_reps=1, dve_pairs=(), gps_copy=False,
                  ablate=(), swi=False, s_split=None):
    """ablate: timing-experiment switches ('ones', 'out', 'act', 'scores')
    that drop parts of the pipeline (results become wrong).
    swi: use DoubleRowSwInterleave (host pre-interleaved weights, contiguous
    LDWEIGHTS stream) instead of DoubleRow."""
    if depth is None:
        depth = DEPTH
    if bufs_e is None:
        bufs_e = max(BUFS_E, depth + 2)
    if s_split is None:
        s_split = S_SPLIT
    return _build(ch, j_total, n_half, loop_reps, bufs_e, depth, hw_reps,
                  frozenset(dve_pairs), gps_copy, frozenset(ablate), swi,
                  s_split)


def _build(ch, j_total, n_half, loop_reps, bufs_e, depth, hw_reps,
           dve_pairs, gps_copy, ablate, swi=False, s_split=False):
    """Emit the per-core Bass/Tile program (SPMD across 8 cores).

    loop_reps: python-unrolled repetitions of the main loop (timing only).
    hw_reps: hardware-loop (For_i) repetitions of the main loop (timing
      only; cheap to compile, so thousands of reps are practical).
    dve_pairs: pair indices whose exp runs on the (otherwise idle) DVE via
      the Schraudolph bit trick instead of ACT — offloads the bottleneck
      engine. Safe here: each column's E feeds both o and s from the same
      tile, so the ~3% fast-exp error on the dominant softmax diagonal
      cancels in the o/s division.
    """
    n_pair = j_total // 256   # j-block pairs (DoubleRow granularity)
    qs = 512                  # output column chunk width (one PSUM bank)
    nq = n_half // qs

    nc = bacc.Bacc("TRN2", target_bir_lowering=False, debug=False, num_devices=8)

    n_jb = j_total // 128
    fs_shape = [128, n_jb, 2, 128] if swi else [128, 2, j_total]
    km_shape = ([128, n_pair, 2, 2, 128] if swi
                else [128, n_pair, 2, ch])
    fs_d = nc.dram_tensor("fs8", fs_shape, FP8E4, kind="ExternalInput").ap()
    f_d = nc.dram_tensor("f8", [128, 2, n_half], FP8E4, kind="ExternalInput").ap()
    km_d = nc.dram_tensor("km8", km_shape, FP8E4, kind="ExternalInput").ap()
    o_d = nc.dram_tensor("o", [ch, n_half], FP16, kind="ExternalOutput").ap()
    s_d = nc.dram_tensor("s", [1, n_half], FP32, kind="ExternalOutput").ap()
    s2_d = (nc.dram_tensor("s2", [128, n_half // 512, 2, 512], FP32,
                           kind="ExternalOutput").ap() if s_split else None)

    with tile.TileContext(nc) as tc, ExitStack() as ctx:
        const_p = ctx.enter_context(tc.tile_pool(name="const", bufs=1))
        fs_p = ctx.enter_context(tc.tile_pool(name="fs", bufs=1))
        f8_p = ctx.enter_context(tc.tile_pool(name="f8", bufs=1))
        km_p = ctx.enter_context(tc.tile_pool(name="km", bufs=1))
        e_p = ctx.enter_context(tc.tile_pool(name="e", bufs=bufs_e))
        osb_p = ctx.enter_context(tc.tile_pool(name="osb", bufs=3))
        ssb_p = ctx.enter_context(tc.tile_pool(name="ssb", bufs=2))
        ps_sc = ctx.enter_context(
            tc.tile_pool(name="ps_sc", bufs=2, space="PSUM"))
        ps_out = ctx.enter_context(
            tc.tile_pool(name="ps_out", bufs=4, space="PSUM"))
        ei_p = (ctx.enter_context(tc.tile_pool(name="ei", bufs=2))
                if dve_pairs else None)
        acc_p = (ctx.enter_context(tc.tile_pool(name="acc", bufs=2))
                 if s_split else None)

        # Schraudolph fast-exp constants: bits(2^t) ~= (t + 127 - c) * 2^23
        FEXP_MUL = 1.4426950408889634 * 8388608.0
        FEXP_ADD = (EXP_BIAS * 1.4426950408889634 + 127.0 - 0.04303) * 8388608.0

        # DoubleRow ldweights needs pair-dim byte-stride % 16 == 0
        ones8 = const_p.tile([128, 2, 16], FP8E4, tag="ones")
        nc.vector.memset(ones8[:], 1.0)
        bias_e = const_p.tile([128, 1], FP32, tag="bias_e")
        nc.vector.memset(bias_e[:], EXP_BIAS)
        # dummy exp: pulls the ACT exp-table load into the DMA window
        warm = const_p.tile([128, 1], FP32, tag="warm")
        nc.scalar.activation(warm[:], bias_e[:],
                             mybir.ActivationFunctionType.Exp)

        fs = fs_p.tile(fs_shape, FP8E4, tag="fs")
        f8 = f8_p.tile([128, 2, n_half], FP8E4, tag="f8")
        km = km_p.tile(km_shape, FP8E4, tag="km")
        PM = mybir.MatmulPerfMode.DoubleRowSwInterleave if swi else DR

        def fs_w(jb):
            return fs[:, jb] if swi else fs[:, :, jb * 128:(jb + 1) * 128]

        def km_w(p0, cb):
            return (km[:, p0, cb] if swi
                    else km[:, p0, :, cb * 128:(cb + 1) * 128])
        e_static = None
        if "act" in ablate:
            e_static = const_p.tile([128, 2, qs], FP8E5, tag="e_static")
            nc.vector.memset(e_static[:], 0.001)

        # input DMA ordered by first-use time; the first critical transfers
        # go out on separate engine queues so their DGE setups overlap
        def fs_dma(eng, jb0, jb1):
            if swi:
                eng.dma_start(out=fs[:, jb0:jb1], in_=fs_d[:, jb0:jb1])
            else:
                eng.dma_start(out=fs[:, :, jb0 * 128:jb1 * 128],
                              in_=fs_d[:, :, jb0 * 128:jb1 * 128])

        nc.sync.dma_start(out=f8[:, :, 0:qs], in_=f_d[:, :, 0:qs])
        fs_dma(nc.gpsimd, 0, 4)
        nc.scalar.dma_start(out=km[:, 0:2], in_=km_d[:, 0:2])
        fs_dma(nc.sync, 4, 8)
        nc.gpsimd.dma_start(out=km[:, 2:6], in_=km_d[:, 2:6])
        fs_dma(nc.sync, 8, 16)
        fs_dma(nc.gpsimd, 16, n_jb)
        nc.gpsimd.dma_start(out=km[:, 6:n_pair], in_=km_d[:, 6:n_pair])
        for q in range(1, nq):
            nc.sync.dma_start(out=f8[:, :, q * qs:(q + 1) * qs],
                              in_=f_d[:, :, q * qs:(q + 1) * qs])

        # fused main loop, globally software-pipelined: score matmuls and
        # exp lead the e-dependent (out/ones) matmuls by `depth` pairs
        # ACROSS chunk boundaries, so the in-order PE queue never drains
        # at a chunk flush and ACT stays fed continuously.
        import contextlib
        loop_ctx = (tc.For_i(0, hw_reps) if hw_reps > 1
                    else contextlib.nullcontext())
        with loop_ctx:
            gps = [(qq, pp) for _ in range(loop_reps)
                   for qq in range(nq) for pp in range(n_pair)]
            N = len(gps)
            etiles = {}
            cur_out = {}
            for idx in range(N + depth):
                if idx < N:
                    q, pp = gps[idx]
                    nsl = slice(q * qs, (q + 1) * qs)
                    ps = ps_sc.tile([128, 2, qs], FP32, tag="sc", name="ps")
                    if "scores" not in ablate:
                        for i in range(2):
                            jb = pp * 2 + i
                            nc.tensor.matmul(
                                ps[:, i, :], fs_w(jb), f8[:, :, nsl],
                                start=True, stop=True, perf_mode=PM,
                            )
                    if "act" in ablate:
                        etiles[idx] = e_static
                    elif pp in dve_pairs:
                        e = e_p.tile([128, 2, qs], FP8E5, tag="e", name="e")
                        ei = ei_p.tile([128, 2, qs], I32, tag="ei", name="ei")
                        nc.vector.tensor_scalar(
                            ei[:], ps[:], FEXP_MUL, FEXP_ADD,
                            mybir.AluOpType.mult, mybir.AluOpType.add,
                        )
                        cpeng = nc.gpsimd if gps_copy else nc.vector
                        cpeng.tensor_copy(e[:], ei[:].bitcast(FP32))
                        etiles[idx] = e
                    else:
                        e = e_p.tile([128, 2, qs], FP8E5, tag="e", name="e")
                        nc.scalar.activation(
                            e[:], ps[:], mybir.ActivationFunctionType.Exp,
                            bias=bias_e[:],
                        )
                        etiles[idx] = e
                if idx >= depth:
                    jdx = idx - depth
                    q2, p2 = gps[jdx]
                    nsl2 = slice(q2 * qs, (q2 + 1) * qs)
                    if p2 == 0:
                        cur_out = {
                            "out": [
                                ps_out.tile([128, qs], FP32, tag="out",
                                            name=f"out_ps{cb}")
                                for cb in range(2)
                            ],
                            "sum": ps_out.tile([1, qs], FP32, tag="out",
                                               name="sum_ps"),
                        }
                    e = etiles.pop(jdx)
                    if "out" not in ablate:
                        for cb in range(2):
                            nc.tensor.matmul(
                                cur_out["out"][cb][:], km_w(p2, cb), e[:],
                                start=(p2 == 0), stop=(p2 == n_pair - 1),
                                perf_mode=PM,
                            )
                    if "ones" not in ablate:
                        if s_split and p2 % 2 == 1:
                            # odd pairs: accumulate s on the idle DVE; the
                            # [128, 2, qs] partial sums are folded on host
                            if p2 == 1:
                                acc = acc_p.tile([128, 2, qs], FP32,
                                                 tag="acc", name="acc")
                                cur_out["acc"] = acc
                                nc.vector.tensor_copy(acc[:], e[:])
                            else:
                                acc = cur_out["acc"]
                                nc.vector.tensor_add(acc[:], acc[:], e[:])
                        else:
                            # ones stays plain DoubleRow: DRS rejects 1-col
                            # weights (s3_lw_valid_num_active_cols)
                            nc.tensor.matmul(
                                cur_out["sum"][:], ones8[:, :, 0:1], e[:],
                                start=(p2 == 0),
                                stop=(p2 == (n_pair - 2 if s_split
                                             else n_pair - 1)),
                                perf_mode=DR,
                            )
                    if p2 == n_pair - 1:
                        last = jdx == N - 1
                        if "ones" not in ablate:
                            srow = ssb_p.tile([1, qs], FP32, tag="srow",
                                              name="srow")
                            nc.vector.tensor_copy(srow[:], cur_out["sum"][:])
                            (nc.gpsimd if last else nc.sync).dma_start(
                                out=s_d[0:1, nsl2], in_=srow[:])
                            if s_split:
                                nc.gpsimd.dma_start(
                                    out=s2_d[:, q2], in_=cur_out["acc"][:])
                        for cb in range(2 if "out" not in ablate else 0):
                            osb = osb_p.tile([128, qs], FP16, tag="osb",
                                             name="osb")
                            if last and cb == 0:
                                # ACT is idle after its final exp; splitting
                                # the drain copies between ACT and DVE
                                # shortens the program tail
                                nc.scalar.copy(osb[:], cur_out["out"][cb][:])
                            else:
                                nc.vector.tensor_copy(
                                    osb[:], cur_out["out"][cb][:])
                            eng = ((nc.scalar if cb == 0 else nc.sync)
                                   if last else nc.sync)
                            eng.dma_start(
                                out=o_d[cb * 128:(cb + 1) * 128, nsl2],
                                in_=osb[:])

    nc.compile()
    return nc


_CACHE = {}


def _get_program():
    if "nc" not in _CACHE:
        _CACHE["nc"] = build_program(swi=USE_SWI, dve_pairs=DVE_PAIRS)
    return _CACHE["nc"]


def _get_runner():
    """Cached sharded executable over 8 cores (same program/plugin as
    run_bass_kernel_spmd's axon path, but without per-call retracing)."""
    if "runner" in _CACHE:
        return _CACHE["runner"]
    import jax
    from jax.sharding import Mesh, NamedSharding, PartitionSpec
    from jax.experimental.shard_map import shard_map
    from concourse import bass2jax, mybir
    from concourse.bass2jax import _bass_exec_p, partition_id_tensor

    nc = _get_program()
    bass2jax.install_neuronx_cc_hook()
    pname = nc.partition_id_tensor.name if nc.partition_id_tensor else None

    in_names, out_names, out_avals = [], [], []
    for alloc in nc.m.functions[0].allocations:
        if not isinstance(alloc, mybir.MemoryLocationSet):
            continue
        name = alloc.memorylocations[0].name
        if alloc.kind == "ExternalInput":
            if name != pname:
                in_names.append(name)
        elif alloc.kind == "ExternalOutput":
            out_names.append(name)
            out_avals.append(
                jax.core.ShapedArray(
                    tuple(alloc.tensor_shape), mybir.dt.np(alloc.dtype)
                )
            )
    n_params, n_outs = len(in_names), len(out_names)
    all_in = in_names + out_names + ([pname] if pname else [])

    def _body(*args):
        operands = list(args)
        if pname is not None:
            operands.append(partition_id_tensor())
        return tuple(_bass_exec_p.bind(
            *operands, out_avals=tuple(out_avals), in_names=tuple(all_in),
            out_names=tuple(out_names), lowering_input_output_aliases=(),
            sim_require_finite=True, sim_require_nnan=True, nc=nc,
        ))

    devices = jax.devices()[:8]
    mesh = Mesh(np.asarray(devices), ("core",))
    spec = NamedSharding(mesh, PartitionSpec("core"))
    fn = jax.jit(
        shard_map(
            _body, mesh=mesh,
            in_specs=(PartitionSpec("core"),) * (n_params + n_outs),
            out_specs=(PartitionSpec("core"),) * n_outs,
            check_rep=False,
        ),
        donate_argnums=tuple(range(n_params, n_params + n_outs)),
        keep_unused=True,
    )
    zero_host = [
        np.zeros((8 * a.shape[0], *a.shape[1:]), a.dtype) for a in out_avals
    ]

    def run(in_maps):
        concat_in = [
            np.concatenate([np.asarray(m[name]) for m in in_maps], axis=0)
            for name in in_names
        ]
        zeros = [jax.device_put(z, spec) for z in zero_host]
        out = fn(*concat_in, *zeros)
        return [
            {
                name: np.asarray(out[i]).reshape(8, *out_avals[i].shape)[c]
                for i, name in enumerate(out_names)
            }
            for c in range(8)
        ]

    _CACHE["runner"] = run
    return run


def _drs_pack(W):
    """SwInterleave weight packing: logical [..., 2, F] -> flat layout with
    A/B pairs interleaved per column and columns reversed (HW-verified)."""
    return np.ascontiguousarray(
        np.flip(W, axis=-1).swapaxes(-1, -2)).reshape(W.shape)


def make_in_maps(foreground, mask, swi=False):
    """Per-core host-side input prep: fp8 casts + DoubleRow layouts."""
    bs, ch, h, w = foreground.shape
    hw = h * w
    half = hw // 2
    f = np.ascontiguousarray(foreground.reshape(bs, ch, hw), dtype=np.float32)
    m = np.ascontiguousarray(mask.reshape(bs, hw), dtype=np.float32)
    in_maps = []
    for b in range(bs):
        k = f[b] + np.float32(1e-7)                 # [ch, hw], reference's +1e-7
        rstd = 1.0 / np.sqrt((k * k).sum(axis=0, dtype=np.float64))  # [hw]
        rstd = rstd.astype(np.float32)
        # scores lhsT: rstd folded in; [128, 2, hw] ch-pair layout
        fs8 = (rstd[None, :] * k).reshape(2, 128, hw).transpose(1, 0, 2)
        fs8 = np.ascontiguousarray(fs8).astype(E4)
        # out lhsT: km[j, c] = rstd_j m_j K[j, c]; [128, pairs, 2, ch]
        km_full = ((rstd * m[b])[:, None] * k.T)     # [hw, ch]
        km8 = km_full.reshape(hw // 256, 2, 128, ch).transpose(2, 0, 1, 3)
        km8 = np.ascontiguousarray(km8).astype(E4)
        if swi:
            # fs: [128, 2, hw] -> per-jb packed [128, n_jb, 2, 128]
            fs8 = _drs_pack(
                fs8.reshape(128, 2, hw // 128, 128).transpose(0, 2, 1, 3)
            )
            # km: [128, pairs, 2, ch] -> per-(pair, cb) packed
            # [128, pairs, cb, 2, 128]
            km8 = _drs_pack(
                km8.reshape(128, hw // 256, 2, 2, 128).transpose(0, 1, 3, 2, 4)
            )
        for hh in range(2):
            fh = f[b][:, hh * half:(hh + 1) * half]  # [ch, half]
            f8 = fh.reshape(2, 128, half).transpose(1, 0, 2)
            in_maps.append({
                "fs8": fs8,
                "f8": np.ascontiguousarray(f8).astype(E4),
                "km8": km8,
            })
    return in_maps


def kernel(foreground, mask):
    foreground = np.asarray(foreground, dtype=np.float32)
    mask = np.asarray(mask, dtype=np.float32)
    bs, ch, h, w = foreground.shape
    hw = h * w

    in_maps = make_in_maps(foreground, mask, swi=USE_SWI)
    try:
        results = _get_runner()(in_maps)
    except Exception:
        # robust fallback: the generic SPMD entry point
        res = run_bass_kernel_spmd(_get_program(), in_maps, list(range(8)))
        results = res.results

    fmap = np.empty((bs, ch, h, w), dtype=np.float32)
    rows = h // 2
    for core in range(8):
        b, hh = core // 2, core % 2
        o = results[core]["o"]       # [ch, hw/2] unnormalized
        s = results[core]["s"]       # [1, hw/2] softmax denominator
        if S_SPLIT:
            s = s + results[core]["s2"].astype(np.float64).sum(
                axis=(0, 2)).reshape(1, -1).astype(np.float32)
        fmap[b, :, hh * rows:(hh + 1) * rows, :] = (o / s).reshape(ch, rows, w)

    mm = mask[:, 0:1]                    # [bs, 1, h, w]
    final = fmap * (1.0 - mm) + foreground * mm
    skip = mask.sum(axis=(1, 2, 3)) > (hw - 10)
    final[skip] = foreground[skip]
    return final.astype(np.float32)
